# revision 7
# baseline (speedup 1.0000x reference)
"""ArcFace logits kernel for 8 trn2 NeuronCores (class-axis model parallel).

kernel(input, weight, label) -> [1024, 100000] f32 scaled-margin logits.

Device work per core (SPMD over 8 cores):
  - x [1024,512] f32 (replicated): row-l2-normalize (x30 scale folded in),
    cast bf16, PE-transpose -> xnT [512,1024] bf16.
  - w shard [12800,512] f32 (12500 real rows + zero pad): stream in 1MB
    tiles, row-l2-normalize, cast bf16, PE-transpose -> wT [512, c] bf16.
  - cosine slab out[n, c] = xnT.T @ wT accumulated f32 in PSUM over 4
    k-chunks, evicted to SBUF (DVE/ACT alternating), DMA'd to DRAM.
Host: concatenate 8 slabs, then apply the ArcFace margin at the 1024
labeled positions using the device-computed cosines (same math as ref).
"""

import math
from contextlib import ExitStack

import numpy as np

import concourse.bass as bass
import concourse.bacc as bacc
import concourse.mybir as mybir
from concourse.masks import make_identity
from concourse.tile import TileContext
from concourse.bass_utils import run_bass_kernel_spmd

F32 = mybir.dt.float32
BF16 = mybir.dt.bfloat16

N = 1024          # batch
D = 512           # in_features
C_TOTAL = 100000  # out_features
N_CORES = 8
C_PER = C_TOTAL // N_CORES     # 12500
C_PAD = 12800                  # 25 chunks of 512, 100 tiles of 128
N_CHUNKS = C_PAD // 512        # 25
NT = N // 128                  # 8 batch tiles
KT = D // 128                  # 4 k-chunks

SCALE = 30.0
MARGIN = 0.5
COS_M = math.cos(MARGIN)
SIN_M = math.sin(MARGIN)
TH = math.cos(math.pi - MARGIN)
MM = math.sin(math.pi - MARGIN) * MARGIN


def build_nc(n=N, d=D, c_pad=C_PAD):
    nt = n // 128
    kt = d // 128
    n_chunks = c_pad // 512

    nc = bacc.Bacc(None, target_bir_lowering=False, debug=False)
    x = nc.declare_dram_parameter("x", [n, d], F32, isOutput=False)
    w = nc.declare_dram_parameter("w", [c_pad, d], F32, isOutput=False)
    out = nc.declare_dram_parameter("out", [n, c_pad], F32, isOutput=True)

    with ExitStack() as ctx:
        tc = ctx.enter_context(TileContext(nc))

        consts = ctx.enter_context(tc.tile_pool(name="consts", bufs=1))
        xpool = ctx.enter_context(tc.tile_pool(name="xpool", bufs=2))
        stats = ctx.enter_context(tc.tile_pool(name="stats", bufs=6))
        scr = ctx.enter_context(tc.tile_pool(name="scr", bufs=3))
        xnt_pool = ctx.enter_context(tc.tile_pool(name="xnt", bufs=1))
        wpool = ctx.enter_context(tc.tile_pool(name="wpool", bufs=3))
        wbf_pool = ctx.enter_context(tc.tile_pool(name="wbf", bufs=3))
        wt2_pool = ctx.enter_context(tc.tile_pool(name="wt2", bufs=2))
        opool = ctx.enter_context(tc.tile_pool(name="opool", bufs=6))
        psum = ctx.enter_context(tc.tile_pool(name="psum", space="PSUM", bufs=2))

        ident = consts.tile([128, 128], BF16)
        make_identity(nc, ident[:])
        eps = consts.tile([128, 1], F32)
        nc.vector.memset(eps, 1e-24)

        # ---- phase 1: xnT [k 4x128, n] bf16, with 1/||x|| * SCALE folded in
        xt_ps = [
            psum.tile([128, min(n, 1024)], BF16, tag=f"tp{k % 2}", name=f"xtps{k}")
            for k in range(kt)
        ]
        for b in range(nt):
            x_tile = xpool.tile([128, d], F32, name="x_tile")
            nc.sync.dma_start(out=x_tile, in_=x[b * 128:(b + 1) * 128, :])
            sq = scr.tile([128, d], F32, name="sq")
            ssq = stats.tile([128, 1], F32, name="ssq")
            # ssq = sum((x/SCALE)^2) = ||x||^2 / SCALE^2
            nc.scalar.activation(
                out=sq, in_=x_tile, func=mybir.ActivationFunctionType.Square,
                scale=1.0 / SCALE, accum_out=ssq,
            )
            nrm = stats.tile([128, 1], F32, name="nrm")
            # sqrt(ssq + eps) = ||x|| / SCALE
            nc.scalar.activation(
                out=nrm, in_=ssq, func=mybir.ActivationFunctionType.Sqrt,
                bias=eps[:], scale=1.0,
            )
            rs = stats.tile([128, 1], F32, name="rs")
            nc.vector.reciprocal(out=rs, in_=nrm)
            xbf = scr.tile([128, d], BF16, name="xbf")
            nc.vector.tensor_scalar_mul(xbf, x_tile, rs)
            for k in range(kt):
                nc.tensor.transpose(
                    out=xt_ps[k][:, b * 128:(b + 1) * 128],
                    in_=xbf[:, k * 128:(k + 1) * 128],
                    identity=ident[:],
                )
        xnT = []
        for k in range(kt):
            t = xnt_pool.tile([128, n], BF16, tag=f"xnt{k}", name=f"xnT{k}")
            nc.vector.tensor_copy(t, xt_ps[k])
            xnT.append(t)

        # ---- phase 2: stream W chunks, build wT, matmul, write out
        w_r = w[:].rearrange("(g t p) k -> g p t k", p=128, t=4)
        for g in range(n_chunks):
            wnat = wpool.tile([128, 4, d], F32, name="wnat")
            nc.sync.dma_start(out=wnat, in_=w_r[g])

            wt_ps = [
                psum.tile([128, 1024], BF16, tag=f"tp{j}", name=f"wtps{j}")
                for j in range(2)
            ]
            for t in range(4):
                sqw = scr.tile([128, d], F32, name="sqw")
                ssqw = stats.tile([128, 1], F32, name="ssqw")
                nc.scalar.activation(
                    out=sqw, in_=wnat[:, t], func=mybir.ActivationFunctionType.Square,
                    accum_out=ssqw,
                )
                nrmw = stats.tile([128, 1], F32, name="nrmw")
                nc.scalar.activation(
                    out=nrmw, in_=ssqw, func=mybir.ActivationFunctionType.Sqrt,
                    bias=eps[:], scale=1.0,
                )
                rsw = stats.tile([128, 1], F32, name="rsw")
                nc.vector.reciprocal(out=rsw, in_=nrmw)
                wbf = wbf_pool.tile([128, d], BF16, name="wbf")
                nc.vector.tensor_scalar_mul(wbf, wnat[:, t], rsw)
                for k in range(kt):
                    # wT block [k 128, c 128] at column t*128 of k's chunk strip
                    j, half = k // 2, k % 2
                    nc.tensor.transpose(
                        out=wt_ps[j][:, half * 512 + t * 128: half * 512 + (t + 1) * 128],
                        in_=wbf[:, k * 128:(k + 1) * 128],
                        identity=ident[:],
                    )
            wt_sb = []
            for j in range(2):
                t2 = wt2_pool.tile([128, 1024], BF16, tag=f"wt2_{j}", name=f"wt2_{j}")
                nc.vector.tensor_copy(t2, wt_ps[j])
                wt_sb.append(t2)

            for b in range(nt):
                pt = psum.tile([128, 512], F32, tag="opsum", name="pt", bufs=4)
                for k in range(kt):
                    nc.tensor.matmul(
                        pt,
                        lhsT=xnT[k][:, b * 128:(b + 1) * 128],
                        rhs=wt_sb[k // 2][:, (k % 2) * 512:(k % 2 + 1) * 512],
                        start=(k == 0), stop=(k == kt - 1),
                    )
                ost = opool.tile([128, 512], F32, name="ost")
                if b % 2 == 0:
                    nc.scalar.copy(ost, pt)
                else:
                    nc.vector.tensor_copy(ost, pt)
                nc.sync.dma_start(
                    out=out[b * 128:(b + 1) * 128, g * 512:(g + 1) * 512],
                    in_=ost,
                )
    nc.compile()
    return nc


_NC_CACHE = {}


def _get_nc():
    if "nc" not in _NC_CACHE:
        _NC_CACHE["nc"] = build_nc()
    return _NC_CACHE["nc"]


def prep_in_maps(input, weight):
    x = np.ascontiguousarray(np.asarray(input, dtype=np.float32))
    w = np.asarray(weight, dtype=np.float32).reshape(N_CORES, C_PER, D)
    in_maps = []
    for i in range(N_CORES):
        wp = np.zeros((C_PAD, D), dtype=np.float32)
        wp[:C_PER] = w[i]
        in_maps.append({"x": x, "w": wp})
    return in_maps


def assemble(results, label):
    out = np.empty((N, C_TOTAL), dtype=np.float32)
    for i in range(N_CORES):
        out[:, i * C_PER:(i + 1) * C_PER] = results[i]["out"][:, :C_PER]
    lab = np.asarray(label).astype(np.int64)
    rows = np.arange(N)
    cos_t = out[rows, lab] / np.float32(SCALE)
    sin_t = np.sqrt(np.maximum(1.0 - cos_t * cos_t, 0.0), dtype=np.float32)
    phi = cos_t * np.float32(COS_M) - sin_t * np.float32(SIN_M)
    phi = np.where(cos_t > np.float32(TH), phi, cos_t - np.float32(MM))
    out[rows, lab] = np.float32(SCALE) * phi
    return out


def kernel(input, weight, label):
    nc = _get_nc()
    in_maps = prep_in_maps(input, weight)
    res = run_bass_kernel_spmd(nc, in_maps, list(range(N_CORES)))
    return assemble(res.results, label)


# revision 21
# speedup vs baseline: 1.1134x; 1.1134x over previous
"""ArcFace logits kernel for 8 trn2 NeuronCores (class-axis model parallel).

kernel(input, weight, label) -> [1024, 100000] f32 scaled-margin logits.

Device work per core (SPMD over 8 cores):
  - x [1024,512] f32 (replicated): row-l2-normalize (x30 scale folded in),
    cast bf16, PE-transpose -> xnT [512,1024] bf16.
  - w shard [12544,512] f32 (12500 real rows + zero pad): stream in 1MB
    tiles, row-l2-normalize, cast bf16, PE-transpose -> wT [512, c] bf16.
  - cosine slab out[n, c] = xnT.T @ wT accumulated f32 in PSUM over 4
    k-chunks, evicted to SBUF (ACT/DVE), DMA'd to DRAM ([1024, 12500]).
Host: concatenate 8 slabs, then apply the ArcFace margin at the 1024
labeled positions using the device-computed cosines (same math as ref).

Engine budget per W chunk (512 classes): PE 32 MM + 16 transposes;
DVE sumsq+recip+wT-evict+2 out-evicts; ACT sqrt+6 out-evicts;
GPSIMD W-load DMA (SWDGE ring, separate from out-write HWDGE FIFO)
+ normalize-mult; SP HWDGE out writes.
"""

import math
from contextlib import ExitStack

import numpy as np

import concourse.bass as bass
import concourse.bacc as bacc
import concourse.mybir as mybir
from concourse.masks import make_identity
from concourse.tile import TileContext
from concourse.bass_utils import run_bass_kernel_spmd

F32 = mybir.dt.float32
BF16 = mybir.dt.bfloat16

N = 1024          # batch
D = 512           # in_features
C_TOTAL = 100000  # out_features
N_CORES = 8
C_PER = C_TOTAL // N_CORES     # 12500 real classes per core
C_PAD = 12800                  # 25 chunks of 512 (uniform; partial-width
                               # chunks hang the HW DMA path — see notes)
KT = D // 128                  # 4 k-chunks

SCALE = 30.0
MARGIN = 0.5
COS_M = math.cos(MARGIN)
SIN_M = math.sin(MARGIN)
TH = math.cos(math.pi - MARGIN)
MM = math.sin(math.pi - MARGIN) * MARGIN


def build_nc(n=N, d=D, c_pad=C_PAD, c_out=C_PER, swdge_w=True, use_ttr=False):
    # NOTE: use_ttr=True (nc.vector.tensor_tensor_reduce) passes CoreSim but
    # wedges real HW (NRT_EXEC_UNIT_UNRECOVERABLE) — do not enable.
    # Partial-width W chunks (c_pad not a multiple of 512) also hang HW.
    nt = n // 128
    kt = d // 128
    n_chunks = (c_pad + 511) // 512

    nc = bacc.Bacc(None, target_bir_lowering=False, debug=False)
    x = nc.declare_dram_parameter("x", [n, d], F32, isOutput=False)
    w = nc.declare_dram_parameter("w", [c_pad, d], F32, isOutput=False)
    out = nc.declare_dram_parameter("out", [n, c_out], F32, isOutput=True)

    with ExitStack() as ctx:
        tc = ctx.enter_context(TileContext(nc))

        consts = ctx.enter_context(tc.tile_pool(name="consts", bufs=1))
        xpool = ctx.enter_context(tc.tile_pool(name="xpool", bufs=3))
        stats = ctx.enter_context(tc.tile_pool(name="stats", bufs=16))
        xnt_pool = ctx.enter_context(tc.tile_pool(name="xnt", bufs=1))
        wpool = ctx.enter_context(tc.tile_pool(name="wpool", bufs=6))
        wbf_pool = ctx.enter_context(tc.tile_pool(name="wbf", bufs=6))
        wt2_pool = ctx.enter_context(tc.tile_pool(name="wt2", bufs=4))
        opool = ctx.enter_context(tc.tile_pool(name="opool", bufs=8))
        psum = ctx.enter_context(tc.tile_pool(name="psum", space="PSUM", bufs=2))

        ident = consts.tile([128, 128], BF16)
        make_identity(nc, ident[:])
        eps = consts.tile([128, 1], F32)
        nc.gpsimd.memset(eps, 1e-24)

        def rsqrt_chain(src_tile, scale, tag):
            """[128,1] rscale = 1/sqrt(sumsq(src)*scale) via DVE ttr + ACT sqrt
            + DVE reciprocal. Returns the [128,1] f32 AP."""
            ssq = stats.tile([128, 1], F32, tag=f"ssq{tag}", name=f"ssq{tag}")
            if use_ttr:
                dummy = stats.tile([128, 1], F32, tag=f"dm{tag}", name=f"dm{tag}")
                nc.vector.tensor_tensor_reduce(
                    out=dummy.broadcast_to(src_tile.shape),
                    in0=src_tile, in1=src_tile, scale=scale, scalar=0.0,
                    op0=mybir.AluOpType.mult, op1=mybir.AluOpType.add,
                    accum_out=ssq,
                )
            else:
                sq = stats.tile([128, d], F32, tag=f"sq{tag}", name=f"sq{tag}", bufs=3)
                nc.scalar.activation(
                    out=sq, in_=src_tile, func=mybir.ActivationFunctionType.Square,
                    scale=math.sqrt(scale), accum_out=ssq,
                )
            nrm = stats.tile([128, 1], F32, tag=f"nrm{tag}", name=f"nrm{tag}")
            # sqrt(ssq + tiny): tiny avoids 1/0 on zero-padded rows
            nc.scalar.activation(
                out=nrm, in_=ssq, func=mybir.ActivationFunctionType.Sqrt,
                bias=eps[:], scale=1.0,
            )
            rs = stats.tile([128, 1], F32, tag=f"rs{tag}", name=f"rs{tag}")
            nc.vector.reciprocal(out=rs, in_=nrm)
            return rs

        # ---- phase 1: xnT [k 4x128, n] bf16, with SCALE/||x|| folded in
        xt_ps = [
            psum.tile([128, min(n, 1024)], BF16, tag=f"tp{k % 2}", name=f"xtps{k}")
            for k in range(kt)
        ]
        for b in range(nt):
            x_tile = xpool.tile([128, d], F32, name="x_tile")
            nc.sync.dma_start(out=x_tile, in_=x[b * 128:(b + 1) * 128, :])
            # sumsq of x/SCALE -> rs = SCALE/||x||
            rs = rsqrt_chain(x_tile, 1.0 / (SCALE * SCALE), "x")
            xbf = xpool.tile([128, d], BF16, name="xbf")
            nc.vector.tensor_scalar_mul(xbf, x_tile, rs)
            for k in range(kt):
                nc.tensor.transpose(
                    out=xt_ps[k][:, b * 128:(b + 1) * 128],
                    in_=xbf[:, k * 128:(k + 1) * 128],
                    identity=ident[:],
                )
        xnT = []
        for k in range(kt):
            t = xnt_pool.tile([128, n], BF16, tag=f"xnt{k}", name=f"xnT{k}")
            nc.vector.tensor_copy(t, xt_ps[k])
            xnT.append(t)

        # ---- phase 2: stream W chunks, build wT, matmul, write out
        for g in range(n_chunks):
            c0 = g * 512
            cw = min(512, c_pad - c0)        # chunk class-width (512 or 256)
            tcnt = cw // 128                 # natural 128-row tiles in chunk
            ow = min(512, c_out - c0)        # columns actually written (<=cw)

            wnat = wpool.tile([128, 4, d], F32, name="wnat")
            w_eng = nc.gpsimd if swdge_w else nc.sync
            w_eng.dma_start(
                out=wnat[:, :tcnt, :],
                in_=w[c0:c0 + cw, :].rearrange("(t p) k -> p t k", p=128),
            )

            wt_ps = [
                psum.tile([128, 1024], BF16, tag=f"tp{j}", name=f"wtps{j}")
                for j in range(2)
            ]
            for t in range(tcnt):
                rsw = rsqrt_chain(wnat[:, t], 1.0, "w")
                wbf = wbf_pool.tile([128, d], BF16, name="wbf")
                nc.vector.tensor_scalar_mul(wbf, wnat[:, t], rsw)
                for k in range(kt):
                    j, half = k // 2, k % 2
                    nc.tensor.transpose(
                        out=wt_ps[j][:, half * 512 + t * 128: half * 512 + (t + 1) * 128],
                        in_=wbf[:, k * 128:(k + 1) * 128],
                        identity=ident[:],
                    )
            wt_sb = []
            for j in range(2):
                t2 = wt2_pool.tile([128, 1024], BF16, tag=f"wt2_{j}", name=f"wt2_{j}")
                nc.scalar.copy(t2[:, 0:cw], wt_ps[j][:, 0:cw])
                nc.scalar.copy(t2[:, 512:512 + cw], wt_ps[j][:, 512:512 + cw])
                wt_sb.append(t2)

            for b in range(nt):
                pt = psum.tile([128, 512], F32, tag="opsum", name="pt", bufs=4)
                for k in range(kt):
                    nc.tensor.matmul(
                        pt[:, 0:cw],
                        lhsT=xnT[k][:, b * 128:(b + 1) * 128],
                        rhs=wt_sb[k // 2][:, (k % 2) * 512:(k % 2) * 512 + cw],
                        start=(k == 0), stop=(k == kt - 1),
                    )
                ost = opool.tile([128, 512], F32, name="ost")
                if b % 3 == 2:
                    nc.vector.tensor_copy(ost[:, 0:ow], pt[:, 0:ow])
                else:
                    nc.scalar.copy(ost[:, 0:ow], pt[:, 0:ow])
                nc.sync.dma_start(
                    out=out[b * 128:(b + 1) * 128, c0:c0 + ow],
                    in_=ost[:, 0:ow],
                )
    nc.compile()
    return nc


_NC_CACHE = {}


def _get_nc():
    if "nc" not in _NC_CACHE:
        _NC_CACHE["nc"] = build_nc()
    return _NC_CACHE["nc"]


def prep_in_maps(input, weight):
    x = np.ascontiguousarray(np.asarray(input, dtype=np.float32))
    w = np.asarray(weight, dtype=np.float32).reshape(N_CORES, C_PER, D)
    in_maps = []
    for i in range(N_CORES):
        wp = np.zeros((C_PAD, D), dtype=np.float32)
        wp[:C_PER] = w[i]
        in_maps.append({"x": x, "w": wp})
    return in_maps


def assemble(results, label):
    out = np.empty((N, C_TOTAL), dtype=np.float32)
    for i in range(N_CORES):
        out[:, i * C_PER:(i + 1) * C_PER] = results[i]["out"][:, :C_PER]
    lab = np.asarray(label).astype(np.int64)
    rows = np.arange(N)
    cos_t = out[rows, lab] / np.float32(SCALE)
    sin_t = np.sqrt(np.maximum(1.0 - cos_t * cos_t, 0.0), dtype=np.float32)
    phi = cos_t * np.float32(COS_M) - sin_t * np.float32(SIN_M)
    phi = np.where(cos_t > np.float32(TH), phi, cos_t - np.float32(MM))
    out[rows, lab] = np.float32(SCALE) * phi
    return out


def kernel(input, weight, label):
    nc = _get_nc()
    in_maps = prep_in_maps(input, weight)
    res = run_bass_kernel_spmd(nc, in_maps, list(range(N_CORES)))
    return assemble(res.results, label)
